# revision 35
# baseline (speedup 1.0000x reference)
"""Trainium2 Bass kernel for DerivativeNet (per-pixel 3-tap derivative stencils).

Computation (per batch b, C=1):
  out_x = nmask * (xK0*u[w-1] + xK1*u[w] + xK2*u[w+1])   (zero-padded in W)
  out_y = nmask * (yK0*u[h-1] + yK1*u[h] + yK2*u[h+1])   (zero-padded in H)
  output = stack([out_x, out_y])  -> [2, B, 1, H, W]

Sharding: pure data parallel over B=8 across the 8 NeuronCores (one batch
element per core).

v4: fp16 I/O, all elementwise work on DVE in the 2x perf mode, 8 even row
tiles, short pipeline fill/drain.

- fp16 end-to-end halves per-core HBM traffic to ~21MB (rel err ~5e-4,
  budget 2e-2). Host packs xK/yK/nmask into one [H, 7, W] tensor (plane
  order x0,y0,x1,y1,x2,y2,nm) so the per-tile load is one fully-contiguous
  DMA with 14KB/partition descriptors; output is stored h-major [H, 2, W].
- GpSimd runs NO elementwise ops: a GpSimd tensor_tensor concurrent with
  DVE drops DVE from 2x to ~1/4 rate (shared SBUF ports) — each GpSimd op
  costs more DVE throughput than it contributes. All 12 ops run on DVE,
  where fp16 packed operands hit the 2x mode (~685ns per 1024-col op).
  With GpSimd idle the HAM activity throttle also stays disengaged.
- All of u (zero-padded in H, pre-transposed on the host to [128, 9, W]:
  row r at partition r%128, plane r//128) is preloaded into one SBUF tile
  U[128, 9, W] via fully-contiguous DMAs, split so tile 0 only waits for
  its own planes. This enables n=128 output rows per tile (8 even tiles,
  no runt: DVE op cost is free-size-bound, so a 9th sweep would cost a
  full extra ~8us).
- Row shifts for the h-stencil run on the TensorEngine (shifted-identity
  fp16 matmul, exact): uc[p]=row r0+1+p (p<=126), udn[p]=row r0+2+p
  (p<=125); the 1-2 seam rows accumulate from U[0:2, t+1] via a tiny k=2
  matmul into the same PSUM bank (an SBUF->SBUF DMA copy into partition
  127 crashed the device). ScalarE downcasts PSUM fp32 -> SBUF fp16.
- ucs holds the center row over the full padded width (cols 0 / W+1
  memset to zero) so the three w-taps are full-width reads at column
  offsets 0/1/2 with no narrowed ops for the w-edge zero-padding.
- Tap products land in one q[128, 6, W] tile in (x0,y0,x1,y1,x2,y2) plane
  order so the two tap-sum adds are fused [128,2,W] instructions.
- First and last tiles are processed in two 512-column halves: per-stage
  latency halves, so the pipeline fills/drains in half the time (costs a
  little extra instruction overhead on those two tiles only).
"""

import numpy as np

import concourse.bass as bass
import concourse.bacc as bacc
import concourse.mybir as mybir
from concourse.tile import TileContext
from concourse.bass_utils import run_bass_kernel_spmd

H = 1024
W = 1024
B = 8
N_CORES = 8
ROWS = 128
NT = H // ROWS  # 8 row tiles
F16 = mybir.dt.float16
F32 = mybir.dt.float32

LAST_RESULTS = None  # test.py reads profiling info from here


def _build() -> bass.Bass:
    nc = bacc.Bacc("TRN2", target_bir_lowering=False)
    # u arrives host-transposed as [128, NT+1, W]: u_d[p, t, :] = u2[t*128+p, :]
    # (u2 = u zero-padded in H). This makes the SBUF preload DMA fully
    # contiguous per partition — a `rearrange` of the row-major layout made
    # the DMA source hop 256KB between 2KB descriptors and crawled at
    # ~80GB/s, gating the pipeline start by ~5us.
    u_d = nc.dram_tensor("u", [128, NT + 1, W], F16, kind="ExternalInput")
    k7_d = nc.dram_tensor("k7", [H, 7, W], F16, kind="ExternalInput")
    out_d = nc.dram_tensor("out", [H, 2, W], F16, kind="ExternalOutput")

    # Stationary matrices (lhsT layout: out[p,:] = sum_k S[k,p]*rhs[k,:]):
    #   S1[k,p] = [k==p+1]  -> uc[p]  = u_t[p+1], p<=126   (cols   0..127)
    #   S2[k,p] = [k==p+2]  -> udn[p] = u_t[p+2], p<=125   (cols 128..255)
    #   L1[k,p] = [k==0][p==127]   patch uc[127]  = u_next[0]  (cols 256..383)
    #   L2[k,p] = [k==p-126]       patch udn[126] = u_next[0],
    #                                    udn[127] = u_next[1]  (cols 384..511)
    sdata = np.zeros((128, 512), dtype=np.float16)
    for p in range(127):
        sdata[p + 1, p] = 1.0
    for p in range(126):
        sdata[p + 2, 128 + p] = 1.0
    sdata[0, 256 + 127] = 1.0
    sdata[0, 384 + 126] = 1.0
    sdata[1, 384 + 127] = 1.0
    shift_d = nc.inline_tensor(sdata, name="shiftmat")

    mult = mybir.AluOpType.mult
    add = mybir.AluOpType.add

    with TileContext(nc) as tc:
        with (
            tc.tile_pool(name="io", bufs=3) as io,
            tc.tile_pool(name="sc", bufs=3) as sc,
            tc.tile_pool(name="ps", bufs=2, space="PSUM") as ps,
            tc.tile_pool(name="mini", bufs=1) as mini,
        ):
            s_t = mini.tile([128, 512], F16, name="s_t", tag="s_t")
            nc.sync.dma_start(out=s_t[:, :], in_=shift_d[:, :])

            # whole padded u in SBUF: U[p, t, :] = u2[t*128 + p, :].
            # Split so tile 0 waits only for its own planes (0 and 1).
            U = mini.tile([128, NT + 1, W], F16, name="U", tag="U")
            nc.sync.dma_start(out=U[:, 0:2, :], in_=u_d[:, 0:2, :])
            nc.sync.dma_start(out=U[:, 2:NT, :], in_=u_d[:, 2:NT, :])
            nc.sync.dma_start(out=U[0:2, NT, :], in_=u_d[0:2, NT, :])

            for t in range(NT):
                r0 = t * ROWS
                # first/last tile: two 512-col halves to halve fill/drain
                split = t == 0 or t == NT - 1
                halves = ((0, 512), (512, 512)) if split else ((0, W),)

                # packed taps+mask: planes x0,y0,x1,y1,x2,y2,nm. Tile 0's
                # load is split by plane so the first taps' operands land
                # first (the whole 1.8MB would gate DVE start by ~5us).
                kt = io.tile([128, 7, W], F16, name="kt", tag="kt")
                if t == 0:
                    nc.scalar.dma_start(out=kt[:, 0:2], in_=k7_d[r0 : r0 + ROWS, 0:2])
                    nc.scalar.dma_start(out=kt[:, 2:7], in_=k7_d[r0 : r0 + ROWS, 2:7])
                else:
                    nc.scalar.dma_start(out=kt[:], in_=k7_d[r0 : r0 + ROWS])

                uc_ps = ps.tile([128, W], F32, name="uc_ps", tag="uc_ps")
                udn_ps = ps.tile([128, W], F32, name="udn_ps", tag="udn_ps")
                ucs = sc.tile([128, W + 2], F16, name="ucs", tag="ucs")
                udns = sc.tile([128, W], F16, name="udns", tag="udns")
                q = sc.tile([128, 6, W], F16, name="q", tag="q")
                a1 = sc.tile([128, 2, W], F16, name="a1", tag="a1")
                out_t = io.tile([128, 2, W], F16, name="out_t", tag="out_t")

                # [128,1] edge memsets on the otherwise idle GpSimd (1 elem
                # per partition -> no meaningful SBUF port pressure on DVE)
                nc.gpsimd.memset(ucs[:, 0:1], 0.0)
                nc.gpsimd.memset(ucs[:, W + 1 : W + 2], 0.0)

                # row-shifted copies via TensorE (exact fp16 matmul):
                # uc_ps[p] = u2[r0+1+p], udn_ps[p] = u2[r0+2+p]; the seam
                # rows (p beyond the shift matrix) accumulate from the next
                # row-plane via a tiny k=2 matmul.
                def shift_mm(which, j):
                    sl, pl, dst = which
                    nc.tensor.matmul(
                        dst[:, j : j + 512],
                        s_t[0:128, sl : sl + 128],
                        U[:, t, j : j + 512],
                        start=True,
                        stop=False,
                    )
                    nc.tensor.matmul(
                        dst[:, j : j + 512],
                        s_t[0:2, pl : pl + 128],
                        U[0:2, t + 1, j : j + 512],
                        start=False,
                        stop=True,
                    )

                UC = (0, 256, uc_ps)
                UDN = (128, 384, udn_ps)
                if t == 0:
                    # emit only what each half needs before it: the first
                    # half's ucs copy reads uc_ps[0:513] (x-right +1 col),
                    # so both uc blocks precede it; udn's second block can
                    # wait until half 1. Shortens the cold-start chain.
                    mm_plan = {
                        0: [(UC, 0), (UC, 512), (UDN, 0)],
                        512: [(UDN, 512)],
                    }
                else:
                    mm_plan = {
                        0: [(UC, 0), (UC, 512), (UDN, 0), (UDN, 512)],
                        512: [],
                    }

                for c0, cw in halves:
                    for which, j in mm_plan[c0]:
                        shift_mm(which, j)
                    c1 = c0 + cw
                    # downcast shifted rows to fp16 SBUF on ScalarE (DVE
                    # operands all-fp16-packed -> 2x mode). The ucs copy
                    # extends one column past the half boundary: the x-right
                    # tap of a split-tile half reads ucs[c1+1], which must
                    # not wait for the next half's copy.
                    ch = min(c1 + 1, W)
                    nc.scalar.copy(ucs[:, 1 + c0 : 1 + ch], uc_ps[:, c0:ch])
                    nc.scalar.copy(udns[:, c0:c1], udn_ps[:, c0:c1])

                    # tap products, plane order (x0,y0,x1,y1,x2,y2). The
                    # y-up tap goes first: it needs only the DMA'd inputs
                    # (no PE/ScalarE chain), so DVE starts earliest.
                    nc.vector.tensor_tensor(
                        q[:, 1, c0:c1], kt[:, 1, c0:c1], U[:, t, c0:c1], mult
                    )
                    nc.vector.tensor_tensor(
                        q[:, 0, c0:c1], kt[:, 0, c0:c1], ucs[:, c0:c1], mult
                    )
                    nc.vector.tensor_tensor(
                        q[:, 2, c0:c1], kt[:, 2, c0:c1], ucs[:, 1 + c0 : 1 + c1], mult
                    )
                    nc.vector.tensor_tensor(
                        q[:, 3, c0:c1], kt[:, 3, c0:c1], ucs[:, 1 + c0 : 1 + c1], mult
                    )
                    nc.vector.tensor_tensor(
                        q[:, 4, c0:c1], kt[:, 4, c0:c1], ucs[:, 2 + c0 : 2 + c1], mult
                    )
                    nc.vector.tensor_tensor(
                        q[:, 5, c0:c1], kt[:, 5, c0:c1], udns[:, c0:c1], mult
                    )

                    # fused pairwise tap sums: a1[:,0]=dx, a1[:,1]=dy
                    nc.vector.tensor_tensor(
                        a1[:, :, c0:c1], q[:, 0:2, c0:c1], q[:, 2:4, c0:c1], add
                    )
                    nc.vector.tensor_tensor(
                        a1[:, :, c0:c1], a1[:, :, c0:c1], q[:, 4:6, c0:c1], add
                    )

                    # mask multiply + store (h-major [H, 2, W])
                    nc.vector.tensor_tensor(
                        out_t[:, 0, c0:c1], a1[:, 0, c0:c1], kt[:, 6, c0:c1], mult
                    )
                    nc.vector.tensor_tensor(
                        out_t[:, 1, c0:c1], a1[:, 1, c0:c1], kt[:, 6, c0:c1], mult
                    )
                    nc.sync.dma_start(
                        out=out_d[r0 : r0 + ROWS, :, c0:c1], in_=out_t[:, :, c0:c1]
                    )
    nc.compile()
    return nc


_PROGRAM = None


def _get_program() -> bass.Bass:
    global _PROGRAM
    if _PROGRAM is None:
        _PROGRAM = _build()
    return _PROGRAM


def kernel(u, nmask, xK, yK):
    global LAST_RESULTS
    nc = _get_program()

    u = np.asarray(u)
    nmask = np.asarray(nmask)
    xK = np.asarray(xK)
    yK = np.asarray(yK)

    in_maps = []
    for b in range(B):
        # u2 = u zero-padded in H, pre-transposed to [128, NT+1, W] so the
        # device-side SBUF preload is a fully contiguous DMA.
        u2 = np.zeros((H + 2, W), dtype=np.float16)
        u2[1 : H + 1, :] = u[b, 0]
        u_pad = np.zeros((128, NT + 1, W), dtype=np.float16)
        u_pad[:, 0:NT, :] = u2[0:H].reshape(NT, 128, W).transpose(1, 0, 2)
        u_pad[0:2, NT, :] = u2[H : H + 2]
        k7 = np.empty((H, 7, W), dtype=np.float16)
        k7[:, 0:6:2, :] = xK[b, 0, 0].transpose(1, 0, 2)  # x taps -> planes 0,2,4
        k7[:, 1:6:2, :] = yK[b, 0, :, 0].transpose(1, 0, 2)  # y taps -> planes 1,3,5
        k7[:, 6, :] = nmask[b, 0]
        in_maps.append({"u": u_pad, "k7": k7})

    res = run_bass_kernel_spmd(nc, in_maps, core_ids=list(range(N_CORES)))
    LAST_RESULTS = res

    outs = [r["out"] for r in res.results]  # each [H, 2, W] fp16
    full = np.stack(outs, axis=0).astype(np.float32)  # [B, H, 2, W]
    full = full.transpose(2, 0, 1, 3)  # [2, B, H, W]
    return np.ascontiguousarray(full[:, :, None, :, :])  # [2, B, 1, H, W]
